# revision 2
# baseline (speedup 1.0000x reference)
"""Trainium2 Bass kernel for per-sample channel-modulated 3x3 conv (CoModConv).

Row-direction Winograd F(2,3):
    s = MLP(y)                                    # (B, C_in) style scales
    For each output row pair (2i, 2i+1), padded input rows d = x[2i..2i+3]:
      V0 = d0 - d2, V1 = d1 + d2, V2 = d2 - d1, V3 = d1 - d3   (per ci, col)
      M_p = sum_{ci,kj} (s[ci] * U_p[ci,kj]) x V_p[ci]          (matmuls)
      out[2i]   = M0 + M1 + M2
      out[2i+1] = M1 - M2 - M3
    with U0 = w[ki=0], U1 = (w0+w1+w2)/2, U2 = (w0-w1+w2)/2, U3 = w[ki=2]
    (host-precomputed; the per-ci style scale commutes with the row mix).

This cuts tensor-engine cycles 1.5x vs direct conv: a 16-output-row
super-group needs 4 position-products (4 PSUM banks, 6 accumulating
128x128x512 bf16 matmuls each = 24) instead of direct's 36. The even/odd
row recombines run on DVE (even) and Pool (odd) under the next group's
matmuls; V row-combos are scalar_tensor_tensor ops in the DVE 4x mode.

Data-parallel over batch: 2 samples per NeuronCore, 8 cores.
"""

import numpy as np
import ml_dtypes

B, D_CAT, C_IN, C_OUT, K, H, W = 16, 512, 256, 256, 3, 64, 64
NCORES = 8
BL = B // NCORES          # samples per core (2)
CIT = C_IN // 128         # ci tiles (2)
COT = C_OUT // 128        # co tiles (2)
GW = W + 2                # padded grid width (66)
GH = H + 2                # padded grid height (66)
NI = H // 2               # winograd row-pair tiles per sample (32)
SGR = 8                   # row-pair tiles per super-group (16 output rows)
NSG = NI // SGR           # super-groups per (sample, co-tile) (4)
VCOLS = NI * GW           # cols per V plane (2112)
ROWS0 = 2 * SGR + 3       # grid rows needed by super-group 0 (19)

# packed MLP params, bf16: [ y^T | w0^T | w1^T | w2^T ]
_PY = 0
_PW0 = _PY + 4 * BL
_PW1 = _PW0 + 4 * C_IN
_PW2 = _PW1 + 2 * C_IN
_PTOT = _PW2 + 2 * C_IN
_NBIAS = 3 * CIT

_BF16 = ml_dtypes.bfloat16
_COMPILED = None


def _build():
    import concourse.mybir as mybir
    import concourse.tile as tile
    from concourse import bacc

    bf16 = mybir.dt.bfloat16
    f32 = mybir.dt.float32
    Prelu = mybir.ActivationFunctionType.Prelu
    Alu = mybir.AluOpType

    nc = bacc.Bacc("TRN2", target_bir_lowering=False, debug=False, num_devices=NCORES)

    pp_in = nc.declare_dram_parameter("pp", [128, _PTOT], bf16, isOutput=False)
    bias_in = nc.declare_dram_parameter("bias", [128, _NBIAS], f32, isOutput=False)
    # host-transformed weights: slabs [w_ki0 | (w0+w1+w2)/2 | (w0-w1+w2)/2 | w_ki2],
    # cols = slab*384 + kj*128 + co
    wu_in = nc.declare_dram_parameter("wu", [CIT, COT, 128, 4 * 384], bf16, isOutput=False)
    xb_in = nc.declare_dram_parameter("xb", [BL, CIT, 128, GH * GW], bf16, isOutput=False)
    out_ext = nc.declare_dram_parameter("out", [BL, COT, 128, H * W], f32, isOutput=True)

    with tile.TileContext(nc) as tc:
        with (
            tc.tile_pool(name="const", bufs=1) as cpool,
            tc.tile_pool(name="xpad", bufs=1) as padpool,
            tc.tile_pool(name="vpl", bufs=1) as vpool,
            tc.tile_pool(name="wmod", bufs=1) as wmpool,
            tc.tile_pool(name="och", bufs=4) as opool,
            tc.tile_pool(name="cpsum", bufs=8, space="PSUM") as cpsum,
        ):
            # warm the scalar-engine activation table before the params land
            warm = cpool.tile([128, 1], f32)
            nc.vector.memset(warm[:], 0.0)
            nc.scalar.activation(warm[:], warm[:], Prelu, bias=warm[:], scale=1.0, alpha=0.01)

            # ---- DMAs: ALL on the sync/HWDGE queue so the serial DMA-engine
            # stream runs in exactly this priority order (SWDGE issues would
            # overtake the HWDGE prefix), and Pool stays free for combines.
            pp_sb = cpool.tile([128, _PTOT], bf16)
            bias_sb = cpool.tile([128, _NBIAS], f32)
            grids = {}
            for b in range(BL):
                for ci_t in range(CIT):
                    grids[(b, ci_t)] = padpool.tile(
                        [128, GH * GW], bf16, name=f"g_{b}_{ci_t}", tag=f"g{b}{ci_t}"
                    )
            wu_sbs = {}
            for co_t in range(COT):
                for ci_t in range(CIT):
                    wu_sbs[(ci_t, co_t)] = cpool.tile(
                        [128, 4 * 384], bf16, name=f"wu_{ci_t}_{co_t}", tag=f"wu{ci_t}{co_t}"
                    )

            nc.sync.dma_start(pp_sb[:, :_PW1], pp_in[:, :_PW1])       # y + w0
            nc.sync.dma_start(
                grids[(0, 0)][:, : ROWS0 * GW], xb_in[0, 0][:, : ROWS0 * GW]
            )
            nc.sync.dma_start(bias_sb[:], bias_in[:])
            nc.sync.dma_start(pp_sb[:, _PW1:], pp_in[:, _PW1:])       # w1 + w2
            nc.sync.dma_start(wu_sbs[(0, 0)][:], wu_in[0, 0])
            nc.sync.dma_start(
                grids[(0, 1)][:, : ROWS0 * GW], xb_in[0, 1][:, : ROWS0 * GW]
            )
            nc.sync.dma_start(wu_sbs[(1, 0)][:], wu_in[1, 0])
            for ci_t in range(CIT):                       # rest of sample-0 rows
                nc.sync.dma_start(
                    grids[(0, ci_t)][:, ROWS0 * GW :], xb_in[0, ci_t][:, ROWS0 * GW :]
                )
            for ci_t in range(CIT):                       # co1 slabs
                nc.sync.dma_start(wu_sbs[(ci_t, 1)][:], wu_in[ci_t, 1])
            for ci_t in range(CIT):                       # sample-1 grids
                nc.sync.dma_start(grids[(1, ci_t)][:], xb_in[1, ci_t])

            # ---- style MLP (fp32): s^T per ci-tile in SBUF ----
            def mlp_layer(rhs_of_kt, kts, w_base, bias_ap, out_sb):
                for ct in range(CIT):
                    mps = cpsum.tile([128, 512], f32, name=f"mlps_{w_base}_{ct}", tag="cps")
                    for kt in range(kts):
                        nc.tensor.matmul(
                            mps[:, :BL],
                            pp_sb[:, w_base + kt * C_IN + ct * 128 :][:, :128],
                            rhs_of_kt(kt),
                            start=(kt == 0),
                            stop=(kt == kts - 1),
                        )
                    nc.scalar.activation(
                        out_sb[:, ct * BL : (ct + 1) * BL],
                        mps[:, :BL],
                        Prelu,
                        bias=bias_ap(ct),
                        scale=1.0,
                        alpha=0.01,
                    )

            s0_sb = cpool.tile([128, CIT * BL], bf16)
            s1_sb = cpool.tile([128, CIT * BL], bf16)
            s_sb = cpool.tile([128, CIT * BL], f32)
            mlp_layer(
                lambda kt: pp_sb[:, _PY + kt * BL : _PY + (kt + 1) * BL],
                4, _PW0, lambda ct: bias_sb[:, ct : ct + 1], s0_sb,
            )
            mlp_layer(
                lambda kt: s0_sb[:, kt * BL : (kt + 1) * BL],
                2, _PW1, lambda ct: bias_sb[:, CIT + ct : CIT + ct + 1], s1_sb,
            )
            mlp_layer(
                lambda kt: s1_sb[:, kt * BL : (kt + 1) * BL],
                2, _PW2, lambda ct: bias_sb[:, 2 * CIT + ct : 2 * CIT + ct + 1], s_sb,
            )

            # ---- V planes: row combos as scalar_tensor_tensor (DVE 4x) ----
            vts = {}

            def v_tile(b, ci_t):
                if (b, ci_t) not in vts:
                    vts[(b, ci_t)] = vpool.tile([128, 4 * VCOLS], bf16, name=f"vt_{b}_{ci_t}", tag=f"v{b}{ci_t}")
                return vts[(b, ci_t)]

            def v_chunk(b, ci_t, i0, i1, planes=(0, 1, 2, 3)):
                """Emit V plane row-pair tiles i0..i1 for (b, ci_t)."""
                # grid rows r as (i, parity): row 2i+tw = g3[:, i, tw, :]
                g3 = grids[(b, ci_t)][:].rearrange(
                    "p (i tw c) -> p i tw c", tw=2, c=GW
                )
                vt = v_tile(b, ci_t)[:].rearrange("p (q i c) -> p q i c", i=NI, c=GW)
                n = i1 - i0
                d0 = g3[:, i0 : i0 + n, 0]          # rows 2i
                d1 = g3[:, i0 : i0 + n, 1]          # rows 2i+1
                d2 = g3[:, i0 + 1 : i0 + 1 + n, 0]  # rows 2i+2
                d3 = g3[:, i0 + 1 : i0 + 1 + n, 1]  # rows 2i+3
                # plain tensor_tensor add/sub runs in the DVE 2x bf16 mode
                combos = {
                    0: (d0, d2, Alu.subtract),
                    1: (d1, d2, Alu.add),
                    2: (d2, d1, Alu.subtract),
                    3: (d1, d3, Alu.subtract),
                }
                for q in planes:
                    a, bb, op = combos[q]
                    nc.vector.tensor_tensor(vt[:, q, i0:i1], a, bb, op)

            # ---- modulated weights: one 4x tensor_scalar per (b, ci_t, co_t) ----
            w_mods = {}

            def mods(b, ci_t, co_t):
                t = wmpool.tile([128, 4 * 384], bf16, name=f"wm_{b}_{ci_t}_{co_t}", tag=f"m{b}{ci_t}{co_t}")
                nc.vector.tensor_scalar_mul(
                    t[:],
                    wu_sbs[(ci_t, co_t)][:],
                    s_sb[:, ci_t * BL + b : ci_t * BL + b + 1],
                )
                w_mods[(b, ci_t, co_t)] = t

            # startup order on DVE, interleaved so the first chain's deps
            # (V ci0 + mods000, then V ci1 plane-by-plane) resolve earliest
            v_chunk(0, 0, 0, SGR)
            mods(0, 0, 0)
            v_chunk(0, 1, 0, SGR, planes=(0,))
            mods(0, 1, 0)
            v_chunk(0, 1, 0, SGR, planes=(1, 2, 3))
            for ci_t in range(CIT):
                v_chunk(0, ci_t, SGR, NI)

            # ---- conv: per (b, co_t, 16-row supergroup): 4 position-product
            # PSUM tiles of 6 accumulating matmuls each; even rows recombine
            # on DVE, odd rows on Act+Pool; one chunk store per supergroup ----
            def supergroup(b, co_t, i0, ni, last=False):
                key = f"{b}_{co_t}_{i0}"
                nm = ni * W
                ps = [
                    cpsum.tile([128, 512], f32, name=f"cps_{key}_{p}", tag="cps")
                    for p in range(4)
                ]
                # in the last group, finish p0 LAST so only the two even
                # combines trail the final matmul (odd path completes early)
                for p in ((1, 2, 3, 0) if last else range(4)):
                    q = 0
                    for ci_t in range(CIT):
                        vt = v_tile(b, ci_t)[:].rearrange(
                            "p (q i c) -> p q i c", i=NI, c=GW
                        )
                        wm = w_mods[(b, ci_t, co_t)]
                        for kj in range(K):
                            nc.tensor.matmul(
                                ps[p][:, :nm],
                                wm[:, p * 384 + kj * 128 : p * 384 + (kj + 1) * 128],
                                vt[:, p, i0 : i0 + ni, kj : kj + W],
                                start=(q == 0),
                                stop=(q == 2 * K - 1),
                            )
                            q += 1
                # vector ops may read only ONE PSUM operand: stage M1/M2 into
                # SBUF via the (otherwise idle) scalar engine, then combine
                # in-place into the output chunk
                ch = opool.tile([128, ni * 2 * W], f32, name=f"och_{key}", tag="och")
                chv = ch[:].rearrange("p (i e c) -> p i e c", e=2, c=W)
                m1 = opool.tile([128, nm], f32, name=f"m1_{key}", tag="tsg1")
                m2 = opool.tile([128, nm], f32, name=f"m2_{key}", tag="tsg2")
                nc.scalar.copy(m1[:], ps[1][:, :nm])
                nc.scalar.copy(m2[:], ps[2][:, :nm])
                if last:
                    # tail: odd path on DVE and emitted first; even ops (the
                    # only consumers of the final p0 chain) use SBUF m2
                    nc.vector.tensor_tensor(chv[:, :, 1], m1[:], m2[:], Alu.subtract)
                    nc.vector.tensor_tensor(
                        chv[:, :, 1], chv[:, :, 1], ps[3][:, :nm], Alu.subtract
                    )
                    nc.vector.tensor_tensor(chv[:, :, 0], ps[0][:, :nm], m1[:], Alu.add)
                    nc.vector.tensor_tensor(chv[:, :, 0], chv[:, :, 0], m2[:], Alu.add)
                else:
                    nc.vector.tensor_tensor(chv[:, :, 0], ps[0][:, :nm], m1[:], Alu.add)
                    nc.vector.tensor_tensor(
                        chv[:, :, 0], chv[:, :, 0], ps[2][:, :nm], Alu.add
                    )
                    m3n = opool.tile([128, nm], f32, name=f"m3n_{key}", tag="tsg3")
                    nc.scalar.activation(
                        m3n[:], ps[3][:, :nm], mybir.ActivationFunctionType.Copy, scale=-1.0
                    )
                    nc.gpsimd.tensor_tensor(chv[:, :, 1], m1[:], m2[:], Alu.subtract)
                    nc.gpsimd.tensor_tensor(chv[:, :, 1], chv[:, :, 1], m3n[:], Alu.add)
                nc.sync.dma_start(
                    out_ext[b, co_t][:, i0 * 2 * W : (i0 + ni) * 2 * W], ch[:]
                )

            for b in range(BL):
                for co_t in range(COT):
                    for sg in range(NSG):
                        final = b == BL - 1 and co_t == COT - 1 and sg == NSG - 1
                        if final:
                            # taper the last supergroup so the tail after the
                            # final matmul is one small combine + 1KB store
                            supergroup(b, co_t, sg * SGR, SGR // 2)
                            supergroup(b, co_t, sg * SGR + SGR // 2, SGR // 4)
                            supergroup(b, co_t, sg * SGR + 3 * SGR // 4, SGR // 4, last=True)
                        else:
                            supergroup(b, co_t, sg * SGR, SGR)
                        # interleave DVE prep for upcoming windows
                        if b == 0 and co_t == 0 and sg == 0:
                            for ci_t in range(CIT):
                                mods(0, ci_t, 1)
                        if b == 0 and co_t == 1:
                            if sg < CIT:
                                v_chunk(1, sg, 0, NI)
                            elif sg == 2:
                                for ci_t in range(CIT):
                                    mods(1, ci_t, 0)
                            elif sg == 3:
                                for ci_t in range(CIT):
                                    mods(1, ci_t, 1)

    nc.compile()
    return nc


def _get_nc():
    global _COMPILED
    if _COMPILED is None:
        _COMPILED = _build()
    return _COMPILED


def _prep_in_maps(x, y, w0, b0, w1, b1, w2, b2, conv_w):
    x = np.ascontiguousarray(x, dtype=np.float32)
    y = np.ascontiguousarray(y, dtype=np.float32)

    # packed per-core-invariant MLP params (bf16), biases fp32
    pp_shared = np.empty((128, _PTOT), dtype=_BF16)
    pp_shared[:, _PW0 : _PW0 + 4 * C_IN] = (
        w0.astype(np.float32).T.reshape(4, 128, C_IN).transpose(1, 0, 2).reshape(128, 4 * C_IN)
    ).astype(_BF16)
    pp_shared[:, _PW1 : _PW1 + 2 * C_IN] = (
        w1.astype(np.float32).T.reshape(2, 128, C_IN).transpose(1, 0, 2).reshape(128, 2 * C_IN)
    ).astype(_BF16)
    pp_shared[:, _PW2 : _PW2 + 2 * C_IN] = (
        w2.astype(np.float32).T.reshape(2, 128, C_IN).transpose(1, 0, 2).reshape(128, 2 * C_IN)
    ).astype(_BF16)
    bias = np.empty((128, _NBIAS), dtype=np.float32)
    for i, bb in enumerate((b0, b1, b2)):
        bias[:, i * CIT : (i + 1) * CIT] = bb.astype(np.float32).reshape(CIT, 128).T

    # winograd row-transformed conv weights:
    # w: (co_t, co, ci_t, ci, ki, kj) -> slabs [w0 | (w0+w1+w2)/2 | (w0-w1+w2)/2 | w2]
    # laid out (ci_t, co_t, ci, slab, kj, co)
    w6 = conv_w.astype(np.float32).reshape(COT, 128, CIT, 128, K, K)
    slabs = np.stack(
        [
            w6[..., 0, :],
            (w6[..., 0, :] + w6[..., 1, :] + w6[..., 2, :]) * 0.5,
            (w6[..., 0, :] - w6[..., 1, :] + w6[..., 2, :]) * 0.5,
            w6[..., 2, :],
        ],
        axis=4,
    )  # (co_t, co, ci_t, ci, slab, kj)
    wu = np.ascontiguousarray(
        slabs.transpose(2, 0, 3, 4, 5, 1).reshape(CIT, COT, 128, 4 * 384)
    ).astype(_BF16)

    xb_all = np.zeros((B, CIT, 128, GH, GW), dtype=_BF16)
    xb_all[:, :, :, 1 : H + 1, 1 : W + 1] = x.reshape(B, CIT, 128, H, W)
    xb_all = xb_all.reshape(B, CIT, 128, GH * GW)

    in_maps = []
    for c in range(NCORES):
        sl = slice(c * BL, (c + 1) * BL)
        pp = pp_shared.copy()
        pp[:, _PY : _PY + 4 * BL] = (
            y[sl].T.reshape(4, 128, BL).transpose(1, 0, 2).reshape(128, 4 * BL)
        ).astype(_BF16)
        in_maps.append(
            {
                "pp": pp,
                "bias": bias,
                "wu": wu,
                "xb": np.ascontiguousarray(xb_all[sl]),
            }
        )
    return in_maps


def _run(in_maps, trace=False):
    from concourse.bass_utils import run_bass_kernel_spmd

    nc = _get_nc()
    res = run_bass_kernel_spmd(nc, in_maps, list(range(NCORES)), trace=trace)
    out = np.concatenate(
        [res.results[c]["out"].reshape(BL, C_OUT, H, W) for c in range(NCORES)], axis=0
    ).astype(np.float32, copy=False)
    return out, res


def kernel(x, y, w0, b0, w1, b1, w2, b2, conv_w):
    in_maps = _prep_in_maps(x, y, w0, b0, w1, b1, w2, b2, conv_w)
    out, _ = _run(in_maps, trace=False)
    return out


# revision 3
# speedup vs baseline: 1.0003x; 1.0003x over previous
"""Trainium2 Bass kernel for per-sample channel-modulated 3x3 conv (CoModConv).

Row-direction Winograd F(2,3):
    s = MLP(y)                                    # (B, C_in) style scales
    For each output row pair (2i, 2i+1), padded input rows d = x[2i..2i+3]:
      V0 = d0 - d2, V1 = d1 + d2, V2 = d2 - d1, V3 = d1 - d3   (per ci, col)
      M_p = sum_{ci,kj} (s[ci] * U_p[ci,kj]) x V_p[ci]          (matmuls)
      out[2i]   = M0 + M1 + M2
      out[2i+1] = M1 - M2 - M3
    with U0 = w[ki=0], U1 = (w0+w1+w2)/2, U2 = (w0-w1+w2)/2, U3 = w[ki=2]
    (host-precomputed; the per-ci style scale commutes with the row mix).

This cuts tensor-engine cycles 1.5x vs direct conv: a 16-output-row
super-group needs 4 position-products (4 PSUM banks, 6 accumulating
128x128x512 bf16 matmuls each = 24) instead of direct's 36. The even/odd
row recombines run on DVE (even) and Pool (odd) under the next group's
matmuls; V row-combos are tensor_tensor ops in the DVE 2x bf16 mode.

Data-parallel over batch: 2 samples per NeuronCore, 8 cores.
"""

import numpy as np
import ml_dtypes

B, D_CAT, C_IN, C_OUT, K, H, W = 16, 512, 256, 256, 3, 64, 64
NCORES = 8
BL = B // NCORES          # samples per core (2)
CIT = C_IN // 128         # ci tiles (2)
COT = C_OUT // 128        # co tiles (2)
GW = W + 2                # padded grid width (66)
GH = H + 2                # padded grid height (66)
NI = H // 2               # winograd row-pair tiles per sample (32)
SGR = 8                   # row-pair tiles per super-group (16 output rows)
NSG = NI // SGR           # super-groups per (sample, co-tile) (4)
VCOLS = NI * GW           # cols per V plane (2112)
ROWS0 = 2 * SGR + 3       # grid rows needed by super-group 0 (19)

# packed MLP params, bf16: [ y^T | w0^T | w1^T | w2^T ]
_PY = 0
_PW0 = _PY + 4 * BL
_PW1 = _PW0 + 4 * C_IN
_PW2 = _PW1 + 2 * C_IN
_PTOT = _PW2 + 2 * C_IN
_NBIAS = 3 * CIT

_BF16 = ml_dtypes.bfloat16
_COMPILED = None


def _build():
    import concourse.mybir as mybir
    import concourse.tile as tile
    from concourse import bacc

    bf16 = mybir.dt.bfloat16
    f32 = mybir.dt.float32
    Prelu = mybir.ActivationFunctionType.Prelu
    Alu = mybir.AluOpType

    nc = bacc.Bacc("TRN2", target_bir_lowering=False, debug=False, num_devices=NCORES)

    pp_in = nc.declare_dram_parameter("pp", [128, _PTOT], bf16, isOutput=False)
    bias_in = nc.declare_dram_parameter("bias", [128, _NBIAS], f32, isOutput=False)
    # host-transformed weights: slabs [w_ki0 | (w0+w1+w2)/2 | (w0-w1+w2)/2 | w_ki2],
    # cols = slab*384 + kj*128 + co
    wu_in = nc.declare_dram_parameter("wu", [CIT, COT, 128, 4 * 384], bf16, isOutput=False)
    xb_in = nc.declare_dram_parameter("xb", [BL, CIT, 128, GH * GW], bf16, isOutput=False)
    out_ext = nc.declare_dram_parameter("out", [BL, COT, 128, H * W], f32, isOutput=True)

    with tile.TileContext(nc) as tc:
        with (
            tc.tile_pool(name="const", bufs=1) as cpool,
            tc.tile_pool(name="xpad", bufs=1) as padpool,
            tc.tile_pool(name="vpl", bufs=1) as vpool,
            tc.tile_pool(name="wmod", bufs=1) as wmpool,
            tc.tile_pool(name="och", bufs=4) as opool,
            tc.tile_pool(name="cpsum", bufs=8, space="PSUM") as cpsum,
        ):
            # warm the scalar-engine activation table before the params land
            warm = cpool.tile([128, 1], f32)
            nc.vector.memset(warm[:], 0.0)
            nc.scalar.activation(warm[:], warm[:], Prelu, bias=warm[:], scale=1.0, alpha=0.01)

            # ---- DMAs: ALL on the sync/HWDGE queue so the serial DMA-engine
            # stream runs in exactly this priority order (SWDGE issues would
            # overtake the HWDGE prefix), and Pool stays free for combines.
            pp_sb = cpool.tile([128, _PTOT], bf16)
            bias_sb = cpool.tile([128, _NBIAS], f32)
            grids = {}
            for b in range(BL):
                for ci_t in range(CIT):
                    grids[(b, ci_t)] = padpool.tile(
                        [128, GH * GW], bf16, name=f"g_{b}_{ci_t}", tag=f"g{b}{ci_t}"
                    )
            wu_sbs = {}
            for co_t in range(COT):
                for ci_t in range(CIT):
                    wu_sbs[(ci_t, co_t)] = cpool.tile(
                        [128, 4 * 384], bf16, name=f"wu_{ci_t}_{co_t}", tag=f"wu{ci_t}{co_t}"
                    )

            nc.sync.dma_start(pp_sb[:, :_PW1], pp_in[:, :_PW1])       # y + w0
            nc.sync.dma_start(
                grids[(0, 0)][:, : ROWS0 * GW], xb_in[0, 0][:, : ROWS0 * GW]
            )
            nc.sync.dma_start(bias_sb[:], bias_in[:])
            nc.sync.dma_start(pp_sb[:, _PW1:], pp_in[:, _PW1:])       # w1 + w2
            nc.sync.dma_start(wu_sbs[(0, 0)][:], wu_in[0, 0])
            nc.sync.dma_start(
                grids[(0, 1)][:, : ROWS0 * GW], xb_in[0, 1][:, : ROWS0 * GW]
            )
            nc.sync.dma_start(wu_sbs[(1, 0)][:], wu_in[1, 0])
            for ci_t in range(CIT):                       # rest of sample-0 rows
                nc.sync.dma_start(
                    grids[(0, ci_t)][:, ROWS0 * GW :], xb_in[0, ci_t][:, ROWS0 * GW :]
                )
            for ci_t in range(CIT):                       # co1 slabs
                nc.sync.dma_start(wu_sbs[(ci_t, 1)][:], wu_in[ci_t, 1])
            for ci_t in range(CIT):                       # sample-1 grids
                nc.sync.dma_start(grids[(1, ci_t)][:], xb_in[1, ci_t])

            # ---- style MLP (fp32): s^T per ci-tile in SBUF ----
            def mlp_layer(rhs_of_kt, kts, w_base, bias_ap, out_sb):
                for ct in range(CIT):
                    mps = cpsum.tile([128, 512], f32, name=f"mlps_{w_base}_{ct}", tag="cps")
                    for kt in range(kts):
                        nc.tensor.matmul(
                            mps[:, :BL],
                            pp_sb[:, w_base + kt * C_IN + ct * 128 :][:, :128],
                            rhs_of_kt(kt),
                            start=(kt == 0),
                            stop=(kt == kts - 1),
                        )
                    nc.scalar.activation(
                        out_sb[:, ct * BL : (ct + 1) * BL],
                        mps[:, :BL],
                        Prelu,
                        bias=bias_ap(ct),
                        scale=1.0,
                        alpha=0.01,
                    )

            s0_sb = cpool.tile([128, CIT * BL], bf16)
            s1_sb = cpool.tile([128, CIT * BL], bf16)
            s_sb = cpool.tile([128, CIT * BL], f32)
            mlp_layer(
                lambda kt: pp_sb[:, _PY + kt * BL : _PY + (kt + 1) * BL],
                4, _PW0, lambda ct: bias_sb[:, ct : ct + 1], s0_sb,
            )
            mlp_layer(
                lambda kt: s0_sb[:, kt * BL : (kt + 1) * BL],
                2, _PW1, lambda ct: bias_sb[:, CIT + ct : CIT + ct + 1], s1_sb,
            )
            mlp_layer(
                lambda kt: s1_sb[:, kt * BL : (kt + 1) * BL],
                2, _PW2, lambda ct: bias_sb[:, 2 * CIT + ct : 2 * CIT + ct + 1], s_sb,
            )

            # ---- V planes: row combos as scalar_tensor_tensor (DVE 4x) ----
            vts = {}

            def v_tile(b, ci_t):
                if (b, ci_t) not in vts:
                    vts[(b, ci_t)] = vpool.tile([128, 4 * VCOLS], bf16, name=f"vt_{b}_{ci_t}", tag=f"v{b}{ci_t}")
                return vts[(b, ci_t)]

            def v_chunk(b, ci_t, i0, i1, planes=(0, 1, 2, 3)):
                """Emit V plane row-pair tiles i0..i1 for (b, ci_t)."""
                # grid rows r as (i, parity): row 2i+tw = g3[:, i, tw, :]
                g3 = grids[(b, ci_t)][:].rearrange(
                    "p (i tw c) -> p i tw c", tw=2, c=GW
                )
                vt = v_tile(b, ci_t)[:].rearrange("p (q i c) -> p q i c", i=NI, c=GW)
                n = i1 - i0
                d0 = g3[:, i0 : i0 + n, 0]          # rows 2i
                d1 = g3[:, i0 : i0 + n, 1]          # rows 2i+1
                d2 = g3[:, i0 + 1 : i0 + 1 + n, 0]  # rows 2i+2
                d3 = g3[:, i0 + 1 : i0 + 1 + n, 1]  # rows 2i+3
                # plain tensor_tensor add/sub runs in the DVE 2x bf16 mode
                combos = {
                    0: (d0, d2, Alu.subtract),
                    1: (d1, d2, Alu.add),
                    2: (d2, d1, Alu.subtract),
                    3: (d1, d3, Alu.subtract),
                }
                for q in planes:
                    a, bb, op = combos[q]
                    nc.vector.tensor_tensor(vt[:, q, i0:i1], a, bb, op)

            # ---- modulated weights: one 4x tensor_scalar per (b, ci_t, co_t) ----
            w_mods = {}

            def mods(b, ci_t, co_t):
                t = wmpool.tile([128, 4 * 384], bf16, name=f"wm_{b}_{ci_t}_{co_t}", tag=f"m{b}{ci_t}{co_t}")
                nc.vector.tensor_scalar_mul(
                    t[:],
                    wu_sbs[(ci_t, co_t)][:],
                    s_sb[:, ci_t * BL + b : ci_t * BL + b + 1],
                )
                w_mods[(b, ci_t, co_t)] = t

            # startup order on DVE, interleaved so the first chain's deps
            # (V ci0 + mods000, then V ci1 plane-by-plane) resolve earliest
            v_chunk(0, 0, 0, SGR)
            mods(0, 0, 0)
            v_chunk(0, 1, 0, SGR, planes=(0,))
            mods(0, 1, 0)
            v_chunk(0, 1, 0, SGR, planes=(1, 2, 3))
            for ci_t in range(CIT):
                v_chunk(0, ci_t, SGR, NI)

            # ---- conv: per (b, co_t, 16-row supergroup): 4 position-product
            # PSUM tiles of 6 accumulating matmuls each; even rows recombine
            # on DVE, odd rows on Act+Pool; one chunk store per supergroup ----
            def supergroup(b, co_t, i0, ni, last=False):
                key = f"{b}_{co_t}_{i0}"
                nm = ni * W
                ps = [
                    cpsum.tile([128, 512], f32, name=f"cps_{key}_{p}", tag="cps")
                    for p in range(4)
                ]
                # in the last group, finish p0 LAST so only the two even
                # combines trail the final matmul (odd path completes early)
                for p in ((1, 2, 3, 0) if last else range(4)):
                    q = 0
                    for ci_t in range(CIT):
                        vt = v_tile(b, ci_t)[:].rearrange(
                            "p (q i c) -> p q i c", i=NI, c=GW
                        )
                        wm = w_mods[(b, ci_t, co_t)]
                        for kj in range(K):
                            nc.tensor.matmul(
                                ps[p][:, :nm],
                                wm[:, p * 384 + kj * 128 : p * 384 + (kj + 1) * 128],
                                vt[:, p, i0 : i0 + ni, kj : kj + W],
                                start=(q == 0),
                                stop=(q == 2 * K - 1),
                            )
                            q += 1
                # vector ops may read only ONE PSUM operand: stage M1/M2 into
                # SBUF via the (otherwise idle) scalar engine, then combine
                # in-place into the output chunk
                ch = opool.tile([128, ni * 2 * W], f32, name=f"och_{key}", tag="och")
                chv = ch[:].rearrange("p (i e c) -> p i e c", e=2, c=W)
                m1 = opool.tile([128, nm], f32, name=f"m1_{key}", tag="tsg1")
                m2 = opool.tile([128, nm], f32, name=f"m2_{key}", tag="tsg2")
                nc.scalar.copy(m1[:], ps[1][:, :nm])
                nc.scalar.copy(m2[:], ps[2][:, :nm])
                if last:
                    # tail: odd path on DVE and emitted first; even ops (the
                    # only consumers of the final p0 chain) use SBUF m2
                    nc.vector.tensor_tensor(chv[:, :, 1], m1[:], m2[:], Alu.subtract)
                    nc.vector.tensor_tensor(
                        chv[:, :, 1], chv[:, :, 1], ps[3][:, :nm], Alu.subtract
                    )
                    nc.vector.tensor_tensor(chv[:, :, 0], ps[0][:, :nm], m1[:], Alu.add)
                    nc.vector.tensor_tensor(chv[:, :, 0], chv[:, :, 0], m2[:], Alu.add)
                else:
                    nc.vector.tensor_tensor(chv[:, :, 0], ps[0][:, :nm], m1[:], Alu.add)
                    nc.vector.tensor_tensor(
                        chv[:, :, 0], chv[:, :, 0], ps[2][:, :nm], Alu.add
                    )
                    m3n = opool.tile([128, nm], f32, name=f"m3n_{key}", tag="tsg3")
                    nc.scalar.activation(
                        m3n[:], ps[3][:, :nm], mybir.ActivationFunctionType.Copy, scale=-1.0
                    )
                    nc.gpsimd.tensor_tensor(chv[:, :, 1], m1[:], m2[:], Alu.subtract)
                    nc.gpsimd.tensor_tensor(chv[:, :, 1], chv[:, :, 1], m3n[:], Alu.add)
                nc.sync.dma_start(
                    out_ext[b, co_t][:, i0 * 2 * W : (i0 + ni) * 2 * W], ch[:]
                )

            for b in range(BL):
                for co_t in range(COT):
                    for sg in range(NSG):
                        final = b == BL - 1 and co_t == COT - 1 and sg == NSG - 1
                        if final:
                            # taper the last supergroup so the tail after the
                            # final matmul is one small combine + 1KB store
                            supergroup(b, co_t, sg * SGR, SGR // 2)
                            supergroup(b, co_t, sg * SGR + SGR // 2, SGR // 4)
                            supergroup(b, co_t, sg * SGR + 3 * SGR // 4, SGR // 4, last=True)
                        else:
                            supergroup(b, co_t, sg * SGR, SGR)
                        # interleave DVE prep for upcoming windows
                        if b == 0 and co_t == 0 and sg == 0:
                            for ci_t in range(CIT):
                                mods(0, ci_t, 1)
                        if b == 0 and co_t == 1:
                            if sg < CIT:
                                v_chunk(1, sg, 0, NI)
                            elif sg == 2:
                                for ci_t in range(CIT):
                                    mods(1, ci_t, 0)
                            elif sg == 3:
                                for ci_t in range(CIT):
                                    mods(1, ci_t, 1)

    nc.compile()
    return nc


def _get_nc():
    global _COMPILED
    if _COMPILED is None:
        _COMPILED = _build()
    return _COMPILED


def _prep_in_maps(x, y, w0, b0, w1, b1, w2, b2, conv_w):
    x = np.ascontiguousarray(x, dtype=np.float32)
    y = np.ascontiguousarray(y, dtype=np.float32)

    # packed per-core-invariant MLP params (bf16), biases fp32
    pp_shared = np.empty((128, _PTOT), dtype=_BF16)
    pp_shared[:, _PW0 : _PW0 + 4 * C_IN] = (
        w0.astype(np.float32).T.reshape(4, 128, C_IN).transpose(1, 0, 2).reshape(128, 4 * C_IN)
    ).astype(_BF16)
    pp_shared[:, _PW1 : _PW1 + 2 * C_IN] = (
        w1.astype(np.float32).T.reshape(2, 128, C_IN).transpose(1, 0, 2).reshape(128, 2 * C_IN)
    ).astype(_BF16)
    pp_shared[:, _PW2 : _PW2 + 2 * C_IN] = (
        w2.astype(np.float32).T.reshape(2, 128, C_IN).transpose(1, 0, 2).reshape(128, 2 * C_IN)
    ).astype(_BF16)
    bias = np.empty((128, _NBIAS), dtype=np.float32)
    for i, bb in enumerate((b0, b1, b2)):
        bias[:, i * CIT : (i + 1) * CIT] = bb.astype(np.float32).reshape(CIT, 128).T

    # winograd row-transformed conv weights:
    # w: (co_t, co, ci_t, ci, ki, kj) -> slabs [w0 | (w0+w1+w2)/2 | (w0-w1+w2)/2 | w2]
    # laid out (ci_t, co_t, ci, slab, kj, co)
    w6 = conv_w.astype(np.float32).reshape(COT, 128, CIT, 128, K, K)
    slabs = np.stack(
        [
            w6[..., 0, :],
            (w6[..., 0, :] + w6[..., 1, :] + w6[..., 2, :]) * 0.5,
            (w6[..., 0, :] - w6[..., 1, :] + w6[..., 2, :]) * 0.5,
            w6[..., 2, :],
        ],
        axis=4,
    )  # (co_t, co, ci_t, ci, slab, kj)
    wu = np.ascontiguousarray(
        slabs.transpose(2, 0, 3, 4, 5, 1).reshape(CIT, COT, 128, 4 * 384)
    ).astype(_BF16)

    xb_all = np.zeros((B, CIT, 128, GH, GW), dtype=_BF16)
    xb_all[:, :, :, 1 : H + 1, 1 : W + 1] = x.reshape(B, CIT, 128, H, W)
    xb_all = xb_all.reshape(B, CIT, 128, GH * GW)

    in_maps = []
    for c in range(NCORES):
        sl = slice(c * BL, (c + 1) * BL)
        pp = pp_shared.copy()
        pp[:, _PY : _PY + 4 * BL] = (
            y[sl].T.reshape(4, 128, BL).transpose(1, 0, 2).reshape(128, 4 * BL)
        ).astype(_BF16)
        in_maps.append(
            {
                "pp": pp,
                "bias": bias,
                "wu": wu,
                "xb": np.ascontiguousarray(xb_all[sl]),
            }
        )
    return in_maps


def _run(in_maps, trace=False):
    from concourse.bass_utils import run_bass_kernel_spmd

    nc = _get_nc()
    res = run_bass_kernel_spmd(nc, in_maps, list(range(NCORES)), trace=trace)
    out = np.concatenate(
        [res.results[c]["out"].reshape(BL, C_OUT, H, W) for c in range(NCORES)], axis=0
    ).astype(np.float32, copy=False)
    return out, res


def kernel(x, y, w0, b0, w1, b1, w2, b2, conv_w):
    in_maps = _prep_in_maps(x, y, w0, b0, w1, b1, w2, b2, conv_w)
    out, _ = _run(in_maps, trace=False)
    return out
